# revision 44
# baseline (speedup 1.0000x reference)
"""MoE soft-routing MLP kernel for 8 Trainium2 NeuronCores.

Reference computation (per layer l, weights a_l: [E, out, in], bias b_l: [E, out]):
    y_e = H @ a_e^T + b_e          # per-expert GEMM      [B, out]
    H'  = sum_e wb[e, :, None] * y_e                      [B, out]
    H'  = elu(H') for layers 0, 1

Distribution: data-parallel over batch B=4096 across 8 cores (B_loc=512).
Expert weights are replicated to every core; x and weight_blend are sharded
along batch.

Per-core algorithm (all activations kept TRANSPOSED on chip: [feature, batch]):
    out[o, b] = sum_e sum_i aT_e[i, o] * (wb[e, b] * Ht[i, b])
  - each expert's contribution accumulates into the same PSUM bank:
    lhsT = aT_e[i-tile, o-chunk] (128x128 stationary),
    rhs  = Zt_e[i-tile] = Ht[i-tile] * bcast(wb[e, :]) (128x512 moving),
  - blend-weight broadcast tiles wbb[e] ([128, B_LOC]) are produced ON CHIP:
    a tiny [E, B_LOC] f32 DMA, then 8 K=1 matmuls (ones[1,128]^T (x) wb[e])
    into PSUM during the cold-clock window — this doubles as the PE HAM
    warmup (the clock gate needs ~3.4us of sustained activity to reach
    2.4GHz) and replaces a 2MB host-broadcast DMA that used to clog the
    early DMA-issue budget. PSUM scratch banks are recycled into L0's
    accumulators via the pool rotation; evictions go 6 on ACT / 2 on DVE
    so every bank frees just before L0's j-outer claims it.
  - ELU+1 is evicted as relu(x) + min(exp(x), 1)  (= elu(x) + 1; valid since
    the preactivations here are far below exp-overflow), and the -1 folds
    into the next layer's blend: zt = (h1 - 1) * wbb_e, one DVE op.

Matmul operands are fp16 with fp32 PSUM accumulation (1 PE cycle/row and
half the weight-DMA bytes of fp32; weights are pre-scaled by 2^8 and blend
weights by 2^6 on the host so fp16 products stay clear of the subnormal
range). x^T is shipped fp16 (halves the startup-critical xt DMA) and the
final layer is evicted UNSCALED to fp16 (PSUM values are out*2^14 ~ [1e-4,
1.1], comfortably normal fp16); the 2^-14 descale happens on the host.
A float32r (TF32-like) fallback is selectable via BASS_MM_MODE=f32r.

Other measured-on-hw details this kernel leans on:
  - only sync (SP), scalar (Activation) and gpsimd can issue DMAs; sync and
    scalar SHARE the 8 HWDGE hardware queues (DMAHW0-7) while gpsimd owns
    8 separate SWDGE queues (DMASW0-7). One dma_start occupies ONE queue
    (~20GB/s) until it completes (the next dma on that queue carries a
    wait on its completion semaphore), so peak DMA bandwidth requires
    loading BOTH pools: the weight stream alternates experts between sync
    (even) and gpsimd (odd), and startup-critical tiles are partition-split
    (32-aligned splits only — odd boundaries corrupt the transfer).
  - one dma_start costs ~640ns of issue time on sync/scalar, ~1us on
    gpsimd (SWDGE); gpsimd's end-of-program SWDGE drain takes ~9us and
    gates the final all-engine barrier, so gpsimd gets NO work that lands
    near the kernel tail (L2 last-expert weights and all output stores go
    to sync/scalar).
  - total DMA (~34MB/core) and PE work (~221us of N=512 matmuls): PE-bound;
    everything else is scheduled to keep the PE gapless (measured residual
    PE idle ~1.5us).

The output of the final layer is DMA'd out still transposed ([512, 512] per
core, fp16, x2^14) and un-transposed + descaled on the host.
"""

import os
import sys

if "/opt/trn_rl_repo" not in sys.path:
    sys.path.insert(0, "/opt/trn_rl_repo")

import numpy as np

import concourse.bass as bass  # noqa: F401  (bass must import before mybir use)
import concourse.mybir as mybir
import concourse.tile as tile
from concourse import bacc
from concourse.bass_utils import run_bass_kernel_spmd

F32 = mybir.dt.float32
F32R = mybir.dt.float32r
F16 = mybir.dt.float16
AF = mybir.ActivationFunctionType
ALU = mybir.AluOpType

# Matmul operand dtype: "f32r" (TF32-like, fp32 bytes in DRAM) or "f16"
# (half the weight DMA; weights pre-scaled by 2^WEXP and blend weights by
# 2^ZEXP on the host to stay out of fp16-subnormal range; the 2^-(WEXP+ZEXP)
# descale is applied on the host after the fp16 output lands).
MM_MODE = os.environ.get("BASS_MM_MODE", "f16")
MM_DT = F16 if MM_MODE == "f16" else F32R
OUT_DT = F16 if MM_MODE == "f16" else F32
WEXP, ZEXP = (8, 6) if MM_MODE == "f16" else (0, 0)
DESCALE = float(2.0 ** -(WEXP + ZEXP))

B, E = 4096, 8
DIMS = [512, 1024, 1024, 512]
N_CORES = 8
B_LOC = B // N_CORES  # 512; also the matmul moving free-dim (max for 4-byte)
P = 128

# (in, out, apply_elu) per layer
LAYERS = [
    (DIMS[0], DIMS[1], True),
    (DIMS[1], DIMS[2], True),
    (DIMS[2], DIMS[3], False),
]

LAST_RESULTS = None  # BassKernelResults of the most recent run (for test.py)
_NC_CACHE = {}


def _build(has_bias):
    """Build the per-core module. has_bias=False (the case this problem's
    setup_inputs actually produces — all beta fills are zeros) drops the
    blended-bias matmuls and their beta/wb feeds entirely; the bank then
    closes on the last expert's product."""
    nc = bacc.Bacc(None, target_bir_lowering=False, debug=False)

    xt = nc.dram_tensor("xt", [DIMS[0], B_LOC], MM_DT, kind="ExternalInput")
    # [1, 128] ones (the K=1 broadcast stationary) followed by the E wb
    # rows, all in one partition — f32r so the BIR verifier accepts them
    # as matmul operands (DMA from an f32r DRAM tensor counts as rounded)
    wba = nc.dram_tensor("wba", [1, P + E * B_LOC], F32R, kind="ExternalInput")
    ats = [
        nc.dram_tensor(f"a{l}t", [E, din, dout], MM_DT, kind="ExternalInput")
        for l, (din, dout, _) in enumerate(LAYERS)
    ]
    wb, betas = None, []
    if has_bias:
        wb = nc.dram_tensor("wb", [E, B_LOC], MM_DT, kind="ExternalInput")
        betas = [
            nc.dram_tensor(f"b{l}", [E, dout], MM_DT, kind="ExternalInput")
            for l, (_, dout, _) in enumerate(LAYERS)
        ]
    outt = nc.dram_tensor("outt", [DIMS[3], B_LOC], OUT_DT, kind="ExternalOutput")

    with tile.TileContext(nc) as tc:
        with (
            tc.tile_pool(name="htp", bufs=12) as htp,
            tc.tile_pool(name="ztp", bufs=16) as ztp,
            tc.tile_pool(name="wp", bufs=18) as wp,
            tc.tile_pool(name="wbbp", bufs=8) as wbbp,
            tc.tile_pool(name="consts", bufs=1) as consts,
            tc.tile_pool(name="betap", bufs=2) as betap,
            tc.tile_pool(name="tmp", bufs=3) as tmp,
            tc.tile_pool(name="psp", bufs=8, space="PSUM") as psp,
        ):
            # --- startup ---
            # Only sync (SP), scalar (Activation) and gpsimd can issue DMAs;
            # each dma_start costs ~640ns of issue time and a single ring
            # moves ~19GB/s, so the tiles on the critical path to the first
            # real matmul (~12us) are partition-split across all three
            # queues in a hand-scheduled order (per-engine emission order
            # below IS that engine's program order).
            #
            ht = [
                htp.tile([P, B_LOC], MM_DT, tag="ht", name="ht")
                for _ in range(DIMS[0] // P)
            ]
            pre_w = [
                wp.tile([P, DIMS[1]], MM_DT, tag="w", name="pre_w")
                for _ in range(DIMS[0] // P)
            ]

            def _xt(eng, j, p0, p1):
                eng.dma_start(
                    out=ht[j][p0:p1, :], in_=xt[j * P + p0 : j * P + p1, :]
                )

            def _pw(eng, j, p0, p1):
                eng.dma_start(
                    out=pre_w[j][p0:p1, :],
                    in_=ats[0][0, j * P + p0 : j * P + p1, :],
                )

            # ones + wb rows, flattened into ONE partition so each row can
            # be the moving operand of a K=1 matmul (moving base partition
            # must be 0) — tiny, first on sync (2 splits): it gates the
            # PE-warmup broadcasts, and its queues free again in ~2us
            wba_sb = consts.tile([1, P + E * B_LOC], F32R, tag="wba")
            wba_mid = P + 4 * B_LOC
            nc.sync.dma_start(
                out=wba_sb[0:1, 0:wba_mid], in_=wba[0:1, 0:wba_mid]
            )
            nc.sync.dma_start(
                out=wba_sb[0:1, wba_mid:], in_=wba[0:1, wba_mid:]
            )

            # DMA queue model (measured): sync+scalar SHARE the 8 HWDGE
            # queues (DMAHW0-7, ~20GB/s each, one dma_start occupies a
            # queue until it completes); gpsimd owns 8 separate SWDGE
            # queues (DMASW0-7). Peak bandwidth needs BOTH pools loaded.
            # First HWDGE lap: pw0 4-way + pw1 2-way + xt0 2-way (8 queues
            # exactly); second lap: pw2/pw3/xt1/xt2; then xt3.
            _pw(nc.sync, 0, 0, 32)
            _pw(nc.sync, 0, 32, 64)
            _pw(nc.sync, 0, 64, 96)
            _pw(nc.sync, 0, 96, 128)
            _pw(nc.sync, 1, 0, 64)
            _pw(nc.sync, 1, 64, 128)
            _pw(nc.sync, 2, 0, 32)
            _pw(nc.sync, 2, 32, 64)
            _pw(nc.sync, 2, 64, 96)
            _pw(nc.sync, 2, 96, 128)
            _pw(nc.sync, 3, 0, 32)
            _pw(nc.sync, 3, 32, 64)
            _pw(nc.sync, 3, 64, 96)
            _pw(nc.sync, 3, 96, 128)
            _xt(nc.scalar, 0, 0, 64)
            _xt(nc.scalar, 0, 64, 128)
            _xt(nc.scalar, 1, 0, 64)
            _xt(nc.scalar, 1, 64, 128)
            _xt(nc.gpsimd, 2, 0, 64)
            _xt(nc.gpsimd, 2, 64, 128)
            _xt(nc.gpsimd, 3, 0, 64)
            _xt(nc.gpsimd, 3, 64, 128)

            # blend-weight broadcast: 8 K=1 matmuls on the tensor engine —
            # this IS the PE warmup (f32r, 1 cycle/row at ap>=256).
            # Evictions: e0..e3 on ACT (before its remaining dma issues),
            # e4..e7 on DVE woven between the first zts below; L0-e0's
            # matmul order defers banks 6/7 so the DVE evictions win the
            # race against the pool's bank reuse.
            wbb = []
            bps = []
            for e in range(E):
                ps = psp.tile([P, B_LOC], F32, tag="ps", name="ps_bc")
                nc.tensor.matmul(
                    ps,
                    wba_sb[0:1, 0:P],
                    wba_sb[0:1, P + e * B_LOC : P + (e + 1) * B_LOC],
                    start=True,
                    stop=True,
                )
                bps.append(ps)
                t = wbbp.tile([P, B_LOC], F32, tag="wbb", name="wbb")
                wbb.append(t)
            # bridge the bcast->first-real-matmul seam so the PE HAM clock
            # gate stays engaged (an idle gap resets the 3.4us ramp and the
            # first real matmuls would run at half clock): a few redundant
            # re-broadcasts of e7 into the same scratch bank — identical
            # value, so write order is irrelevant.
            for _ in range(4):
                nc.tensor.matmul(
                    bps[E - 1],
                    wba_sb[0:1, 0:P],
                    wba_sb[0:1, P + (E - 1) * B_LOC : P + E * B_LOC],
                    start=True,
                    stop=True,
                )
            for e in range(4):
                nc.scalar.activation(wbb[e], bps[e], AF.Copy, scale=1.0)

            # wb as [E, B_LOC] tile: rhs of the (end-of-layer) bias matmuls
            wb_all = None
            if has_bias:
                wb_all = consts.tile([E, B_LOC], MM_DT, tag="wb_all")
                nc.gpsimd.dma_start(out=wb_all, in_=wb[:, :])

            # --- layers ---
            for l, (din, dout, use_act) in enumerate(LAYERS):
                ni, no = din // P, dout // P
                beta_sb = None
                if has_bias:
                    beta_sb = betap.tile([E, dout], MM_DT, tag="beta")
                    nc.gpsimd.dma_start(out=beta_sb, in_=betas[l][:, :])

                psums = []
                for _ in range(no):
                    pt = psp.tile([P, B_LOC], F32, tag="ps", name="ps")
                    psums.append(pt)

                if l == 0:
                    # expert 0 hand-woven: zts interleave with the DVE bcast
                    # evictions, and banks 4..7's first matmuls are deferred
                    # past the j0/j1 wave on banks 0..3 so those evictions
                    # win the race against the pool's bank reuse.
                    zt0 = []
                    for j in range(ni):
                        zt = ztp.tile([P, B_LOC], MM_DT, tag="zt", name="zt0")
                        nc.vector.tensor_mul(zt, ht[j], wbb[0])
                        zt0.append(zt)
                        if j == 0:
                            nc.vector.tensor_scalar_mul(wbb[4], bps[4], 1.0)
                        elif j == 1:
                            nc.vector.tensor_scalar_mul(wbb[5], bps[5], 1.0)
                            nc.vector.tensor_scalar_mul(wbb[6], bps[6], 1.0)
                            nc.vector.tensor_scalar_mul(wbb[7], bps[7], 1.0)
                    for cs in ((0, 1, 2, 3), (4, 5, 6, 7)):
                        for j in (0, 1):
                            for c in cs:
                                nc.tensor.matmul(
                                    psums[c],
                                    pre_w[j][:, c * P : (c + 1) * P],
                                    zt0[j],
                                    start=(j == 0),
                                    stop=False,
                                )
                    for j in (2, 3):
                        for c in range(no):
                            nc.tensor.matmul(
                                psums[c],
                                pre_w[j][:, c * P : (c + 1) * P],
                                zt0[j],
                                start=False,
                                stop=False,
                            )

                # accumulate remaining experts 0/1..E-2 j-outer (consumes ht
                # tiles as the previous layer produces them)
                e_first = 1 if l == 0 else 0
                for e in range(e_first, E - 1):
                    for j in range(ni):
                        zt = ztp.tile([P, B_LOC], MM_DT, tag="zt")
                        if l == 0:
                            nc.vector.tensor_mul(zt, ht[j], wbb[e])
                        else:
                            # ht holds elu(x)+1; fold the -1 into the blend
                            nc.vector.scalar_tensor_tensor(
                                zt, ht[j], -1.0, wbb[e], ALU.add, ALU.mult
                            )
                        # The weight stream alternates between the HWDGE
                        # pool (sync) and the SWDGE pool (gpsimd) so both
                        # sets of DMA queues carry it; L0's early experts
                        # additionally split for latency.
                        if l == 0 and e == 1:
                            plan = [(nc.gpsimd, 0, 64), (nc.gpsimd, 64, 128)]
                        elif l == 0 and e == 2:
                            plan = [(nc.sync, 0, 64), (nc.sync, 64, 128)]
                        elif (l == 0 and e in (4, 6)) or (l > 0 and e % 2 == 1):
                            plan = [(nc.gpsimd, 0, 128)]
                        else:
                            plan = [(nc.sync, 0, 128)]
                        at_sb = wp.tile([P, dout], MM_DT, tag="w")
                        for eng, p0, p1 in plan:
                            eng.dma_start(
                                out=at_sb[p0:p1, :],
                                in_=ats[l][e, j * P + p0 : j * P + p1, :],
                            )
                        for c in range(no):
                            nc.tensor.matmul(
                                psums[c],
                                at_sb[:, c * P : (c + 1) * P],
                                zt,
                                start=(e == e_first and j == 0 and l > 0),
                                stop=False,
                            )
                # last expert runs c-outer (bank-by-bank): bank closures —
                # and therefore evictions, next-layer bank reuse, and the
                # final output stores — spread across the last ~ni*no
                # matmuls instead of clustering after the end.
                e = E - 1
                zts, wts = [], []
                for j in range(ni):
                    zt = ztp.tile([P, B_LOC], MM_DT, tag="zt")
                    if l == 0:
                        nc.vector.tensor_mul(zt, ht[j], wbb[e])
                    else:
                        nc.vector.scalar_tensor_tensor(
                            zt, ht[j], -1.0, wbb[e], ALU.add, ALU.mult
                        )
                    zts.append(zt)
                    at_sb = wp.tile([P, dout], MM_DT, tag="w", name="w_last")
                    # e7 follows the odd-expert SWDGE assignment for l==1;
                    # for l==2 it stays on sync so gpsimd's slow end-of-
                    # kernel SWDGE drain starts well before the kernel tail
                    w_eng = nc.gpsimd if l == 1 else nc.sync
                    w_eng.dma_start(
                        out=at_sb, in_=ats[l][e, j * P : (j + 1) * P, :]
                    )
                    wts.append(at_sb)
                for c in range(no):
                    for j in range(ni):
                        nc.tensor.matmul(
                            psums[c],
                            wts[j][:, c * P : (c + 1) * P],
                            zts[j],
                            start=False,
                            stop=(not has_bias and j == ni - 1),
                        )
                    if has_bias:
                        nc.tensor.matmul(
                            psums[c],
                            beta_sb[:, c * P : (c + 1) * P],
                            wb_all,
                            start=False,
                            stop=True,
                        )

                # evict: elu(x)+1 for layers 0/1, direct DMA out for layer 2
                if use_act:
                    new_ht = []
                    for c in range(no):
                        r = tmp.tile([P, B_LOC], F32, tag="relu")
                        x = tmp.tile([P, B_LOC], F32, tag="expz")
                        h = htp.tile([P, B_LOC], F32, tag="ht")
                        nc.scalar.activation(r, psums[c], AF.Relu, scale=DESCALE)
                        nc.scalar.activation(x, psums[c], AF.Exp, scale=DESCALE)
                        # h = min(x, 1) + r  ( = elu + 1 )
                        nc.vector.scalar_tensor_tensor(h, x, 1.0, r, ALU.min, ALU.add)
                        new_ht.append(h)
                    ht = new_ht
                else:
                    half = B_LOC // 2
                    for c in range(no):
                        # evict split along the FREE dim (engine time scales
                        # with free size): ACT takes the low half, DVE the
                        # high half — the tile is out of PSUM in ~one half-op
                        # time. Values stay scaled by 2^14; fp16 range is
                        # fine and the host descales.
                        o = tmp.tile([P, B_LOC], OUT_DT, tag="out")
                        nc.scalar.activation(
                            o[:, 0:half], psums[c][:, 0:half], AF.Copy, scale=1.0
                        )
                        nc.vector.tensor_scalar_mul(
                            o[:, half:], psums[c][:, half:], 1.0
                        )
                        # stores: sync+scalar only — any gpsimd DMA here
                        # would restart its ~9us SWDGE drain at kernel end,
                        # which gates the final all-engine barrier
                        # scalar also runs the ACT evict halves: give it
                        # only one split per early bank so its sequencer
                        # keeps pace with the 1.73us bank cadence
                        last = c == no - 1
                        if last:
                            bank_engs = [nc.sync, nc.scalar, nc.sync, nc.scalar]
                        else:
                            bank_engs = [nc.sync, nc.sync, nc.sync, nc.scalar]
                        step = P // 4
                        for q in range(4):
                            bank_engs[q].dma_start(
                                out=outt[
                                    c * P + q * step : c * P + (q + 1) * step, :
                                ],
                                in_=o[q * step : (q + 1) * step, :],
                            )

    nc.compile()
    return nc


def _maybe_reset_device():
    """Clear stale NRT state on the axon terminal left by a crashed prior
    process. Only safe/needed before this process initializes its jax
    backend, and must run in a subprocess (CDLL'ing the axon .so in-process
    conflicts with jax's own dlopen)."""
    try:
        import jax._src.xla_bridge as xb

        if getattr(xb, "_backends", None):
            return  # backend already live in this process; don't touch it
    except Exception:
        pass
    try:
        import subprocess

        subprocess.run(
            [
                sys.executable,
                "-c",
                "import ctypes; lib = ctypes.CDLL('/opt/axon/libaxon_pjrt.so'); "
                "lib.axon_reset.restype = ctypes.c_int64; lib.axon_reset()",
            ],
            timeout=60,
            capture_output=True,
        )
    except Exception:
        pass


def kernel(x, weight_blend, a0, b0, a1, b1, a2, b2):
    global LAST_RESULTS, _NC_CACHE
    _maybe_reset_device()
    mm_np = np.float16 if MM_MODE == "f16" else np.float32
    x = np.asarray(x, dtype=np.float32)
    weight_blend = np.ascontiguousarray(np.asarray(weight_blend, dtype=np.float32))
    aT = [
        np.ascontiguousarray(
            (np.asarray(a, dtype=np.float32) * float(2.0**WEXP))
            .transpose(0, 2, 1)
            .astype(mm_np)
        )
        for a in (a0, a1, a2)
    ]
    bs = [
        np.ascontiguousarray(
            (np.asarray(b, dtype=np.float32) * float(2.0 ** (WEXP + ZEXP))).astype(
                mm_np
            )
        )
        for b in (b0, b1, b2)
    ]
    has_bias = any(np.any(b) for b in bs)

    if has_bias not in _NC_CACHE:
        _NC_CACHE[has_bias] = _build(has_bias)
    nc = _NC_CACHE[has_bias]

    in_maps = []
    for c in range(N_CORES):
        sl = slice(c * B_LOC, (c + 1) * B_LOC)
        wb_c = np.ascontiguousarray(weight_blend[:, sl]) * float(2.0**ZEXP)
        wba_row = np.concatenate(
            [np.ones(P, np.float32), wb_c.astype(np.float32).ravel()]
        ).reshape(1, -1)
        m = {
            "xt": np.ascontiguousarray(x[sl].T.astype(mm_np)),
            "wba": wba_row,
            "a0t": aT[0],
            "a1t": aT[1],
            "a2t": aT[2],
        }
        if has_bias:
            m["wb"] = wb_c.astype(mm_np)
            m["b0"], m["b1"], m["b2"] = bs
        in_maps.append(m)

    trace = os.environ.get("BASS_KERNEL_TRACE") == "1"
    res = run_bass_kernel_spmd(
        nc, in_maps, core_ids=list(range(N_CORES)), trace=trace
    )
    LAST_RESULTS = res
    return np.concatenate(
        [
            (np.asarray(r["outt"]).astype(np.float32) * DESCALE).T
            for r in res.results
        ],
        axis=0,
    )
